# revision 1
# baseline (speedup 1.0000x reference)
"""Chunked attention Trainium2 Bass kernel.

Problem: B=2, S=8192, HIDDEN=1024, HEADS=16, HEAD_DIM=64, CHUNK=2048,
OVERLAP=128. Sharding: head-parallel x batch-parallel -> 32 (b,h) jobs,
4 per core on 8 cores. Each core computes full-seq chunked attention for
its 4 heads; host slices inputs / reassembles output.

Per-core pipeline (all f32 / float32r):
  - Q,K loaded natural [s,64], PE-transposed to [d,seq] (K in pairs ->
    row-packed QK^T matmuls, 2 concurrent K_c=64 matmuls).
  - S^T[k,q] matmuls into PSUM groups of 3 banks; one ACT exp per group
    (scale=1/8 folded into activation) -> P^T in SBUF (float32r).
  - PV: lhsT=[V|1] (65 cols) stationary, rhs=P^T -> accumulate
    [O^T; l] in one PSUM bank over all k-tiles.
  - Out: copy O'^T to SBUF, PE-transpose 128-col slices back to natural
    [q, 65], reciprocal of l column + tensor_scalar normalize, blend
    the 128-wide chunk-overlap bands, DMA to HBM.
"""

import sys

if '/opt/trn_rl_repo' not in sys.path:
    sys.path.insert(0, '/opt/trn_rl_repo')

import numpy as np

import concourse.bass as bass
import concourse.mybir as mybir
import concourse.tile as tile
from concourse.bass_utils import run_bass_kernel_spmd
from concourse.masks import make_identity

F32 = mybir.dt.float32
F32R = mybir.dt.float32r
EXP = mybir.ActivationFunctionType.Exp

B, S, HIDDEN, HEADS, HD = 2, 8192, 1024, 16, 64
SCALE = 1.0 / 8.0  # 1/sqrt(64)
N_CORES = 8
JOBS = 4  # (b, h) pairs per core
# (q0, Lq, k0, Lk) per chunk; step=1920, overlap=128
CHUNKS = [
    (0, 2048, 0, 2176),
    (1920, 2048, 1792, 2304),
    (3840, 2048, 3712, 2304),
    (5760, 2048, 5632, 2304),
    (7680, 512, 7552, 640),
]
GROUP = 3  # k-tiles per PSUM group (3 banks x 2 buffers + 1 opsum + 1 tp)


def _legalize_waits(nc, max_waits=1):
    """walrus in this config rejects >1 sync-wait per instruction: hoist
    excess waits onto injected same-engine NoOps placed just before."""
    cnt = 0
    for f in nc.m.functions:
        for blk in f.blocks:
            il = blk.instructions
            if not any(
                i.sync_info is not None and i.sync_info.on_wait
                and len(i.sync_info.on_wait) > max_waits for i in il
            ):
                continue
            new = []
            for inst in il:
                si = inst.sync_info
                if si is not None and si.on_wait and len(si.on_wait) > max_waits:
                    waits = list(si.on_wait)
                    spill, keep = waits[:-max_waits], waits[-max_waits:]
                    for w in spill:
                        nop = mybir.InstNoOp(
                            name=f"I-wsplit-{cnt}", ins=[], outs=[])
                        cnt += 1
                        nop.engine = inst.engine
                        nop.sync_info = mybir.SyncInfo(on_wait=[w], on_update=[])
                        new.append(nop)
                    inst.sync_info = mybir.SyncInfo(
                        on_wait=keep, on_update=list(si.on_update or []))
                new.append(inst)
            blk.instructions = new
    return cnt


def _build_nc():
    nc = bass.Bass()
    q_in = nc.declare_dram_parameter("q", [JOBS, S, HD], F32, isOutput=False)
    k_in = nc.declare_dram_parameter("k", [JOBS, S, HD], F32, isOutput=False)
    v_in = nc.declare_dram_parameter("v", [JOBS, S, HD], F32, isOutput=False)
    bw_in = nc.declare_dram_parameter("bw", [128, 2], F32, isOutput=False)
    out = nc.declare_dram_parameter("out", [JOBS, S, HD], F32, isOutput=True)

    with tile.TileContext(nc) as tc:
        with (
            tc.tile_pool(name="const", bufs=1) as cpool,
            tc.tile_pool(name="stage", bufs=2) as stage,      # qtmp/ktmp
            tc.tile_pool(name="ops", bufs=2) as ops,          # qT/kT/vW
            tc.tile_pool(name="probs", bufs=2) as probs,      # pT
            tc.tile_pool(name="opath", bufs=3) as opath,      # oT/o_out/recip
            tc.tile_pool(name="tailp", bufs=2) as tailp,      # prev-tail
            tc.tile_pool(name="spsum", bufs=2, space="PSUM") as spsum,
            tc.tile_pool(name="onepsum", bufs=1, space="PSUM") as onepsum,
            tc.tile_pool(name="tpsum", bufs=1, space="PSUM") as tpsum,
        ):
            ident = cpool.tile([128, 128], F32)
            make_identity(nc, ident)
            ones_f32 = cpool.tile([128, 1], F32)
            nc.vector.memset(ones_f32, 1.0)
            bw = cpool.tile([128, 2], F32)
            nc.sync.dma_start(out=bw, in_=bw_in[:, :])

            for j in range(JOBS):
                prev_tail = None
                for ci, (q0, lq, k0, lk) in enumerate(CHUNKS):
                    nq = lq // 128
                    nk = lk // 128
                    npair = (nk + 1) // 2

                    # ---- load K natural as pair-tiles, transpose ----
                    ktmp = stage.tile([128, nk * 64], F32, tag="ktmp")
                    nc.sync.dma_start(
                        out=ktmp.rearrange("p (t d) -> p t d", d=HD),
                        in_=k_in[j, k0:k0 + lk, :].rearrange(
                            "(t p) d -> p t d", p=128),
                    )
                    kT = ops.tile([128, npair * 128], F32R, tag="kT")
                    for p in range(npair):
                        w = min(128, nk * 64 - p * 128)  # 128 or 64 (odd tail)
                        tp_t = tpsum.tile([128, 128], F32, tag="tp")
                        nc.tensor.transpose(
                            tp_t[0:w, 0:128],
                            ktmp[:, p * 128:p * 128 + w],
                            ident,
                        )
                        nc.vector.tensor_copy(
                            kT[0:w, p * 128:(p + 1) * 128], tp_t[0:w, 0:128])

                    # ---- load Q natural, transpose + duplicate halves ----
                    qtmp = stage.tile([128, nq * 64], F32, tag="qtmp")
                    nc.sync.dma_start(
                        out=qtmp.rearrange("p (t d) -> p t d", d=HD),
                        in_=q_in[j, q0:q0 + lq, :].rearrange(
                            "(t p) d -> p t d", p=128),
                    )
                    qT = ops.tile([128, lq], F32R, tag="qT")
                    for t in range(nq):
                        tp_t = tpsum.tile([128, 128], F32, tag="tp")
                        nc.tensor.transpose(
                            tp_t[0:64, 0:128],
                            qtmp[:, t * 64:(t + 1) * 64],
                            ident,
                        )
                        nc.vector.tensor_copy(
                            qT[0:64, t * 128:(t + 1) * 128], tp_t[0:64, 0:128])
                        nc.vector.tensor_copy(
                            qT[64:128, t * 128:(t + 1) * 128], tp_t[0:64, 0:128])

                    # ---- load V' = [V | ones] per k-tile (65-stride) ----
                    vW = ops.tile([128, nk * 65], F32R, tag="vW")
                    vv = vW.rearrange("p (t e) -> p t e", e=65)
                    nc.sync.dma_start(
                        out=vv[:, :, 0:64],
                        in_=v_in[j, k0:k0 + lk, :].rearrange(
                            "(t p) d -> p t d", p=128).bitcast(F32R),
                    )
                    nc.vector.tensor_copy(
                        vv[:, :, 64], ones_f32.broadcast_to([128, nk]))

                    ngroups = (nk + GROUP - 1) // GROUP
                    for qb in range(lq // 512):
                        qs = slice(qb * 512, qb * 512 + 512)
                        opsum = onepsum.tile([128, 512], F32, tag="opsum")
                        for g in range(ngroups):
                            kts = list(range(g * GROUP, min((g + 1) * GROUP, nk)))
                            sp = spsum.tile([128, 512 * GROUP], F32, tag="sp")
                            # QK^T: S^T[k,q], row-packed via pair halves
                            for i, kt in enumerate(kts):
                                pr, half = kt // 2, kt % 2
                                rows = slice(64 * half, 64 * half + 64)
                                nc.tensor.matmul(
                                    sp[:, i * 512:(i + 1) * 512],
                                    kT[rows, pr * 128:(pr + 1) * 128],
                                    qT[rows, qs],
                                    start=True, stop=True,
                                    tile_position=(64 * half, 0),
                                    skip_group_check=True,
                                )
                            # exp over the whole group (one ACT instr)
                            pT = probs.tile([128, 512 * GROUP], F32R, tag="pT")
                            nw = 512 * len(kts)
                            nc.scalar.activation(
                                pT[:, 0:nw], sp[:, 0:nw], EXP, scale=SCALE)
                            # PV: accumulate [O^T; l] over k-tiles
                            for i, kt in enumerate(kts):
                                nc.tensor.matmul(
                                    opsum[0:65, :],
                                    vW[:, kt * 65:(kt + 1) * 65],
                                    pT[:, i * 512:(i + 1) * 512],
                                    start=(kt == 0), stop=(kt == nk - 1),
                                    skip_group_check=True,
                                )

                        # ---- drain + normalize + blend + store ----
                        oT = opath.tile([65, 512], F32, tag="oT")
                        nc.vector.tensor_copy(oT, opsum[0:65, :])
                        for s4 in range(4):
                            tp_o = tpsum.tile([128, 128], F32, tag="tp")
                            nc.tensor.transpose(
                                tp_o[:, 0:65],
                                oT[:, s4 * 128:(s4 + 1) * 128],
                                ident[0:65, 0:65],
                            )
                            rec = opath.tile([128, 1], F32, tag="rec")
                            nc.vector.reciprocal(rec, tp_o[:, 64:65])
                            o_out = opath.tile([128, 64], F32, tag="oout")
                            nc.vector.tensor_scalar_mul(
                                o_out, tp_o[:, 0:64], rec)

                            gq = q0 + qb * 512 + s4 * 128
                            is_head = ci > 0 and qb == 0 and s4 == 0
                            is_tail = (ci < len(CHUNKS) - 1 and
                                       qb == lq // 512 - 1 and s4 == 3)
                            if is_head:
                                # blend with saved prev-chunk tail
                                t1 = opath.tile([128, 64], F32, tag="t1")
                                nc.vector.tensor_scalar_mul(
                                    t1, prev_tail, bw[:, 0:1])
                                t2 = opath.tile([128, 64], F32, tag="t2")
                                nc.vector.tensor_scalar_mul(
                                    t2, o_out, bw[:, 1:2])
                                o_fin = opath.tile([128, 64], F32, tag="ofin")
                                nc.vector.tensor_add(o_fin, t1, t2)
                                nc.sync.dma_start(
                                    out=out[j, gq:gq + 128, :], in_=o_fin)
                            elif is_tail:
                                nt = tailp.tile([128, 64], F32, tag="tail")
                                nc.vector.tensor_copy(nt, o_out)
                                prev_tail = nt
                            else:
                                nc.sync.dma_start(
                                    out=out[j, gq:gq + 128, :], in_=o_out)

    _legalize_waits(nc)
    return nc


_NC = None


def _get_nc():
    global _NC
    if _NC is None:
        _NC = _build_nc()
    return _NC


def _blend_weights():
    wt = np.linspace(1.0, 0.0, 128).astype(np.float32)  # prev-chunk tail ramp
    wh = np.linspace(0.0, 1.0, 128).astype(np.float32)  # cur-chunk head ramp
    denom = (wt + wh) + np.float32(1e-10)
    bw = np.stack([wt / denom, wh / denom], axis=1).astype(np.float32)
    return np.ascontiguousarray(bw)


def kernel(query, key, value):
    query = np.asarray(query, dtype=np.float32)
    key_ = np.asarray(key, dtype=np.float32)
    value = np.asarray(value, dtype=np.float32)
    nc = _get_nc()
    bw = _blend_weights()

    qh = query.reshape(B, S, HEADS, HD)
    kh = key_.reshape(B, S, HEADS, HD)
    vh = value.reshape(B, S, HEADS, HD)

    in_maps = []
    for c in range(N_CORES):
        jobs = [(g // HEADS, g % HEADS) for g in range(4 * c, 4 * c + 4)]
        in_maps.append({
            "q": np.ascontiguousarray(
                np.stack([qh[b, :, h] for (b, h) in jobs])),
            "k": np.ascontiguousarray(
                np.stack([kh[b, :, h] for (b, h) in jobs])),
            "v": np.ascontiguousarray(
                np.stack([vh[b, :, h] for (b, h) in jobs])),
            "bw": bw,
        })

    res = run_bass_kernel_spmd(nc, in_maps, list(range(N_CORES)))

    out = np.empty((B, S, HIDDEN), dtype=np.float32)
    for c in range(N_CORES):
        oc = res.results[c]["out"]  # [4, S, 64]
        for jj, g in enumerate(range(4 * c, 4 * c + 4)):
            b, h = g // HEADS, g % HEADS
            out[b, :, h * HD:(h + 1) * HD] = oc[jj]
    return out


# revision 3
# speedup vs baseline: 15.1896x; 15.1896x over previous
"""Chunked attention Trainium2 Bass kernel.

Problem: B=2, S=8192, HIDDEN=1024, HEADS=16, HEAD_DIM=64, CHUNK=2048,
OVERLAP=128. Sharding: head-parallel x batch-parallel -> 32 (b,h) jobs,
4 per core on 8 cores. Each core computes full-seq chunked attention for
its 4 heads; the host slices/pre-transposes inputs and reassembles the
output.

Per-core dataflow (fp32 / float32r):
  - Host supplies Q^T and K^T in [d, seq] layout, duplicated across both
    64-partition halves (rows 0:64 == rows 64:128) so K_c=64 matmuls can
    be row-packed in pairs (two concurrent matmuls in the PE array).
  - QK^T: S^T[k,q] float32r matmuls into PSUM groups of 3 banks.
  - One ACT exp per group (scale=1/8 folded into the activation) ->
    P^T in SBUF (float32r; ACT does not actually round).
  - PV: lhsT=[V|1] (65 cols, stationary) accumulates [O^T; l] into one
    PSUM bank over all k-tiles of the chunk.
  - Normalize in O^T layout: rinv = 1/l (DVE), replicated across the 64
    d-partitions with a ones-outer-product matmul, then one DVE multiply.
  - Chunk-overlap bands (128 wide) blended on DVE with host-provided
    replicated ramp tiles; output written transposed [d, seq], host
    transposes back.
"""

import sys

if '/opt/trn_rl_repo' not in sys.path:
    sys.path.insert(0, '/opt/trn_rl_repo')

import numpy as np

import concourse.bass as bass
import concourse.mybir as mybir
import concourse.tile as tile
from concourse.bass_utils import run_bass_kernel_spmd

F32 = mybir.dt.float32
F32R = mybir.dt.float32r
EXP = mybir.ActivationFunctionType.Exp

B, S, HIDDEN, HEADS, HD = 2, 8192, 1024, 16, 64
SCALE = 1.0 / 8.0  # 1/sqrt(64)
N_CORES = 8
JOBS = 4  # (b, h) pairs per core
# (q0, Lq, k0, Lk) per chunk; step=1920, overlap=128
CHUNKS = [
    (0, 2048, 0, 2176),
    (1920, 2048, 1792, 2304),
    (3840, 2048, 3712, 2304),
    (5760, 2048, 5632, 2304),
    (7680, 512, 7552, 640),
]
GROUP = 3  # k-tiles per PSUM group (3 banks x2 bufs + opsum + rep = 8)


def _legalize_waits(nc, max_waits=1):
    """walrus in this config rejects >1 sync-wait per instruction: hoist
    excess waits onto injected same-engine NoOps placed just before."""
    cnt = 0
    for f in nc.m.functions:
        for blk in f.blocks:
            il = blk.instructions
            if not any(
                i.sync_info is not None and i.sync_info.on_wait
                and len(i.sync_info.on_wait) > max_waits for i in il
            ):
                continue
            new = []
            for inst in il:
                si = inst.sync_info
                if si is not None and si.on_wait and len(si.on_wait) > max_waits:
                    waits = list(si.on_wait)
                    spill, keep = waits[:-max_waits], waits[-max_waits:]
                    for w in spill:
                        nop = mybir.InstNoOp(
                            name=f"I-wsplit-{cnt}", ins=[], outs=[])
                        cnt += 1
                        nop.engine = inst.engine
                        nop.sync_info = mybir.SyncInfo(on_wait=[w], on_update=[])
                        new.append(nop)
                    inst.sync_info = mybir.SyncInfo(
                        on_wait=keep, on_update=list(si.on_update or []))
                new.append(inst)
            blk.instructions = new
    return cnt


def _build_nc():
    nc = bass.Bass()
    qt_in = nc.declare_dram_parameter("qt", [JOBS, 128, S], F32, isOutput=False)
    kt_in = nc.declare_dram_parameter("kt", [JOBS, 128, S], F32, isOutput=False)
    v_in = nc.declare_dram_parameter("v", [JOBS, S, HD], F32, isOutput=False)
    bwt_in = nc.declare_dram_parameter("bwt", [64, 256], F32, isOutput=False)
    out = nc.declare_dram_parameter("out", [JOBS, HD, S], F32, isOutput=True)

    with tile.TileContext(nc) as tc:
        with (
            tc.tile_pool(name="const", bufs=1) as cpool,
            tc.tile_pool(name="ops", bufs=2) as ops,          # qT/kT/vW
            tc.tile_pool(name="probs", bufs=2) as probs,      # pT
            tc.tile_pool(name="opath", bufs=3) as opath,      # rinv/o_nrm/...
            tc.tile_pool(name="tailp", bufs=2) as tailp,      # prev-tail
            tc.tile_pool(name="spsum", bufs=2, space="PSUM") as spsum,
            tc.tile_pool(name="onepsum", bufs=1, space="PSUM") as onepsum,
            tc.tile_pool(name="reppsum", bufs=1, space="PSUM") as reppsum,
        ):
            ones_f32 = cpool.tile([128, 1], F32)
            nc.vector.memset(ones_f32, 1.0)
            ones_r = cpool.tile([1, 64], F32R)
            nc.vector.tensor_copy(ones_r, ones_f32[0:1, 0:1].broadcast_to([1, 64]))
            bwt = cpool.tile([64, 256], F32)
            nc.sync.dma_start(out=bwt, in_=bwt_in[:, :])

            for j in range(JOBS):
                prev_tail = None
                for ci, (q0, lq, k0, lk) in enumerate(CHUNKS):
                    nk = lk // 128
                    last_chunk = ci == len(CHUNKS) - 1

                    qT = ops.tile([128, lq], F32R, tag="qT")
                    nc.sync.dma_start(
                        out=qT, in_=qt_in[j, :, q0:q0 + lq].bitcast(F32R))
                    kT = ops.tile([128, lk], F32R, tag="kT")
                    nc.sync.dma_start(
                        out=kT, in_=kt_in[j, :, k0:k0 + lk].bitcast(F32R))
                    vW = ops.tile([128, nk * 65], F32R, tag="vW")
                    vv = vW.rearrange("p (t e) -> p t e", e=65)
                    nc.sync.dma_start(
                        out=vv[:, :, 0:64],
                        in_=v_in[j, k0:k0 + lk, :].rearrange(
                            "(t p) d -> p t d", p=128).bitcast(F32R),
                    )
                    nc.vector.tensor_copy(
                        vv[:, :, 64], ones_f32.broadcast_to([128, nk]))

                    ngroups = (nk + GROUP - 1) // GROUP
                    for qb in range(lq // 512):
                        qs = slice(qb * 512, qb * 512 + 512)
                        opsum = onepsum.tile([128, 512], F32, tag="opsum")
                        for g in range(ngroups):
                            kts = list(range(g * GROUP, min((g + 1) * GROUP, nk)))
                            sp = spsum.tile([128, 512 * GROUP], F32, tag="sp")
                            # QK^T: S^T[k,q]; consecutive k-tiles alternate
                            # row halves -> pairs run concurrently in PE
                            for i, kt in enumerate(kts):
                                rows = slice(64 * (kt % 2), 64 * (kt % 2) + 64)
                                nc.tensor.matmul(
                                    sp[:, i * 512:(i + 1) * 512],
                                    kT[rows, kt * 128:(kt + 1) * 128],
                                    qT[rows, qs],
                                    start=True, stop=True,
                                    tile_position=(64 * (kt % 2), 0),
                                    skip_group_check=True,
                                )
                            pT = probs.tile([128, 512 * GROUP], F32R, tag="pT")
                            nw = 512 * len(kts)
                            nc.scalar.activation(
                                pT[:, 0:nw], sp[:, 0:nw], EXP, scale=SCALE)
                            for i, kt in enumerate(kts):
                                nc.tensor.matmul(
                                    opsum[0:65, :],
                                    vW[:, kt * 65:(kt + 1) * 65],
                                    pT[:, i * 512:(i + 1) * 512],
                                    start=(kt == 0), stop=(kt == nk - 1),
                                    skip_group_check=True,
                                )

                        # ---- normalize in O^T layout ----
                        rinv_f = opath.tile([1, 512], F32, tag="rinvf")
                        nc.vector.reciprocal(rinv_f, opsum[64:65, :])
                        rinv = opath.tile([1, 512], F32R, tag="rinv")
                        nc.vector.tensor_copy(rinv, rinv_f)
                        rep = reppsum.tile([64, 512], F32, tag="rep")
                        nc.tensor.matmul(rep, ones_r, rinv,
                                         start=True, stop=True,
                                         skip_group_check=True)
                        rep_sb = opath.tile([64, 512], F32, tag="repsb")
                        nc.vector.tensor_copy(rep_sb, rep)
                        o_nrm = opath.tile([64, 512], F32, tag="onrm")
                        nc.vector.tensor_mul(o_nrm, opsum[0:64, :], rep_sb)

                        # ---- blend bands / defer tail / store ----
                        gq = q0 + qb * 512
                        is_head = ci > 0 and qb == 0
                        is_tail = (not last_chunk) and qb == lq // 512 - 1
                        lo = 0      # first col to store directly
                        hi = 512    # end col to store
                        if is_head:
                            lo = 128
                            t1 = opath.tile([64, 128], F32, tag="t1")
                            nc.vector.tensor_mul(t1, prev_tail, bwt[:, 0:128])
                            t2 = opath.tile([64, 128], F32, tag="t2")
                            nc.vector.tensor_mul(
                                t2, o_nrm[:, 0:128], bwt[:, 128:256])
                            o_fin = opath.tile([64, 128], F32, tag="ofin")
                            nc.vector.tensor_add(o_fin, t1, t2)
                            nc.sync.dma_start(
                                out=out[j, :, gq:gq + 128], in_=o_fin)
                        if is_tail:
                            hi = 384
                            nt = tailp.tile([64, 128], F32, tag="tail")
                            nc.vector.tensor_copy(nt, o_nrm[:, 384:512])
                            prev_tail = nt
                        nc.sync.dma_start(
                            out=out[j, :, gq + lo:gq + hi],
                            in_=o_nrm[:, lo:hi])

    _legalize_waits(nc)
    return nc


_NC = None


def _get_nc():
    global _NC
    if _NC is None:
        _NC = _build_nc()
    return _NC


def _blend_weights_rep():
    wt = np.linspace(1.0, 0.0, 128).astype(np.float32)  # prev-chunk tail ramp
    wh = np.linspace(0.0, 1.0, 128).astype(np.float32)  # cur-chunk head ramp
    denom = (wt + wh) + np.float32(1e-10)
    a = (wt / denom).astype(np.float32)
    b = (wh / denom).astype(np.float32)
    bwt = np.empty((64, 256), np.float32)
    bwt[:, 0:128] = a[None, :]
    bwt[:, 128:256] = b[None, :]
    return np.ascontiguousarray(bwt)


def make_in_maps(query, key_, value):
    """Host-side prep: per-core slices; Q^T/K^T in [d, seq] layout
    duplicated across both partition halves."""
    qh = query.reshape(B, S, HEADS, HD)
    kh = key_.reshape(B, S, HEADS, HD)
    vh = value.reshape(B, S, HEADS, HD)
    # [B, H, D, S]
    qT = np.ascontiguousarray(qh.transpose(0, 2, 3, 1))
    kT = np.ascontiguousarray(kh.transpose(0, 2, 3, 1))
    bwt = _blend_weights_rep()
    in_maps = []
    for c in range(N_CORES):
        jobs = [(g // HEADS, g % HEADS) for g in range(4 * c, 4 * c + 4)]
        qt_c = np.empty((JOBS, 128, S), np.float32)
        kt_c = np.empty((JOBS, 128, S), np.float32)
        v_c = np.empty((JOBS, S, HD), np.float32)
        for jj, (b, h) in enumerate(jobs):
            qt_c[jj, 0:64] = qT[b, h]
            qt_c[jj, 64:128] = qT[b, h]
            kt_c[jj, 0:64] = kT[b, h]
            kt_c[jj, 64:128] = kT[b, h]
            v_c[jj] = vh[b, :, h]
        in_maps.append({"qt": qt_c, "kt": kt_c, "v": v_c, "bwt": bwt})
    return in_maps


def assemble_out(results):
    out = np.empty((B, S, HIDDEN), dtype=np.float32)
    for c in range(N_CORES):
        oc = results[c]["out"]  # [4, 64, S]
        for jj, g in enumerate(range(4 * c, 4 * c + 4)):
            b, h = g // HEADS, g % HEADS
            out[b, :, h * HD:(h + 1) * HD] = oc[jj].T
    return out


def kernel(query, key, value):
    query = np.asarray(query, dtype=np.float32)
    key_ = np.asarray(key, dtype=np.float32)
    value = np.asarray(value, dtype=np.float32)
    nc = _get_nc()
    in_maps = make_in_maps(query, key_, value)
    res = run_bass_kernel_spmd(nc, in_maps, list(range(N_CORES)))
    return assemble_out(res.results)


# revision 5
# speedup vs baseline: 25.4779x; 1.6773x over previous
"""Chunked attention Trainium2 Bass kernel.

Problem: B=2, S=8192, HIDDEN=1024, HEADS=16, HEAD_DIM=64, CHUNK=2048,
OVERLAP=128. Sharding: head-parallel x batch-parallel -> 32 (b,h) jobs,
4 per core on 8 cores. Each core computes full-seq chunked attention for
its 4 heads; the host slices/pre-transposes inputs and reassembles the
output.

Per-core dataflow (fp32 / float32r):
  - Host supplies Q^T and K^T in [d, seq] layout, duplicated across both
    64-partition halves (rows 0:64 == rows 64:128) so K_c=64 matmuls can
    be row-packed in pairs (two concurrent matmuls in the PE array).
  - QK^T: S^T[k,q] float32r matmuls into PSUM groups of 3 banks.
  - One ACT exp per group (scale=1/8 folded into the activation) ->
    P^T in SBUF (float32r; ACT does not actually round the values).
  - PV: lhsT=[V|1] (65 cols, stationary) accumulates [O^T; l] into one
    PSUM bank over all k-tiles of the chunk.
  - Device returns the UNNORMALIZED per-chunk [O^T; l] (65 rows per
    chunk, concatenated along seq); softmax division and the 128-wide
    overlap-band blending happen on the host in fp32.
"""

import sys

if '/opt/trn_rl_repo' not in sys.path:
    sys.path.insert(0, '/opt/trn_rl_repo')

import numpy as np

import concourse.bass as bass
import concourse.mybir as mybir
import concourse.tile as tile
from concourse.bass_utils import run_bass_kernel_spmd

F32 = mybir.dt.float32
F32R = mybir.dt.float32r
EXP = mybir.ActivationFunctionType.Exp

B, S, HIDDEN, HEADS, HD = 2, 8192, 1024, 16, 64
SCALE = 1.0 / 8.0  # 1/sqrt(64)
N_CORES = 8
JOBS = 4  # (b, h) pairs per core
# (q0, Lq, k0, Lk) per chunk; step=1920, overlap=128
CHUNKS = [
    (0, 2048, 0, 2176),
    (1920, 2048, 1792, 2304),
    (3840, 2048, 3712, 2304),
    (5760, 2048, 5632, 2304),
    (7680, 512, 7552, 640),
]
COLS = [0, 2048, 4096, 6144, 8192]  # chunk col offsets in the out buffer
SQ = 8704  # sum of chunk Lq
GROUP = 3  # k-tiles per S^T PSUM group (3 banks x2 bufs + opsum x2 = 8)


def _legalize_waits(nc, max_waits=1):
    """walrus in this config rejects >1 sync-wait per instruction: hoist
    excess waits onto injected same-engine NoOps placed just before."""
    cnt = 0
    for f in nc.m.functions:
        for blk in f.blocks:
            il = blk.instructions
            if not any(
                i.sync_info is not None and i.sync_info.on_wait
                and len(i.sync_info.on_wait) > max_waits for i in il
            ):
                continue
            new = []
            for inst in il:
                si = inst.sync_info
                if si is not None and si.on_wait and len(si.on_wait) > max_waits:
                    waits = list(si.on_wait)
                    spill, keep = waits[:-max_waits], waits[-max_waits:]
                    for w in spill:
                        nop = mybir.InstNoOp(
                            name=f"I-wsplit-{cnt}", ins=[], outs=[])
                        cnt += 1
                        nop.engine = inst.engine
                        nop.sync_info = mybir.SyncInfo(on_wait=[w], on_update=[])
                        new.append(nop)
                    inst.sync_info = mybir.SyncInfo(
                        on_wait=keep, on_update=list(si.on_update or []))
                new.append(inst)
            blk.instructions = new
    return cnt


def _build_nc(reps=1):
    nc = bass.Bass()
    qt_in = nc.declare_dram_parameter("qt", [JOBS, 128, S], F32, isOutput=False)
    kt_in = nc.declare_dram_parameter("kt", [JOBS, 128, S], F32, isOutput=False)
    v_in = nc.declare_dram_parameter("v", [JOBS, S, HD], F32, isOutput=False)
    out = nc.declare_dram_parameter("out", [JOBS, 65, SQ], F32, isOutput=True)

    with tile.TileContext(nc) as tc:
        with (
            tc.tile_pool(name="const", bufs=1) as cpool,
            tc.tile_pool(name="ops", bufs=2) as ops,          # qT/kT/vW
            tc.tile_pool(name="probs", bufs=3) as probs,      # pT
            tc.tile_pool(name="opath", bufs=3) as opath,      # o_sb staging
            tc.tile_pool(name="spsum", bufs=2, space="PSUM") as spsum,
            tc.tile_pool(name="onepsum", bufs=2, space="PSUM") as onepsum,
        ):
            ones_f32 = cpool.tile([128, 1], F32)
            nc.vector.memset(ones_f32, 1.0)

            for j in [jj for _ in range(reps) for jj in range(JOBS)]:
                for ci, (q0, lq, k0, lk) in enumerate(CHUNKS):
                    nk = lk // 128

                    qT = ops.tile([128, lq], F32R, tag="qT")
                    nc.sync.dma_start(
                        out=qT, in_=qt_in[j, :, q0:q0 + lq].bitcast(F32R))
                    kT = ops.tile([128, lk], F32R, tag="kT")
                    nc.sync.dma_start(
                        out=kT, in_=kt_in[j, :, k0:k0 + lk].bitcast(F32R))
                    vW = ops.tile([128, nk * 65], F32R, tag="vW")
                    vv = vW.rearrange("p (t e) -> p t e", e=65)
                    nc.sync.dma_start(
                        out=vv[:, :, 0:64],
                        in_=v_in[j, k0:k0 + lk, :].rearrange(
                            "(t p) d -> p t d", p=128).bitcast(F32R),
                    )
                    nc.vector.tensor_copy(
                        vv[:, :, 64], ones_f32.broadcast_to([128, nk]))

                    ngroups = (nk + GROUP - 1) // GROUP
                    for qb in range(lq // 512):
                        qs = slice(qb * 512, qb * 512 + 512)
                        opsum = onepsum.tile([128, 512], F32, tag="opsum")
                        for g in range(ngroups):
                            kts = list(range(g * GROUP, min((g + 1) * GROUP, nk)))
                            sp = spsum.tile([128, 512 * GROUP], F32, tag="sp")
                            # QK^T: S^T[k,q]; consecutive k-tiles alternate
                            # row halves -> pairs run concurrently in PE
                            for i, kt in enumerate(kts):
                                rows = slice(64 * (kt % 2), 64 * (kt % 2) + 64)
                                nc.tensor.matmul(
                                    sp[:, i * 512:(i + 1) * 512],
                                    kT[rows, kt * 128:(kt + 1) * 128],
                                    qT[rows, qs],
                                    start=True, stop=True,
                                    tile_position=(64 * (kt % 2), 0),
                                    skip_group_check=True,
                                )
                            pT = probs.tile([128, 512 * GROUP], F32R, tag="pT")
                            nw = 512 * len(kts)
                            nc.scalar.activation(
                                pT[:, 0:nw], sp[:, 0:nw], EXP, scale=SCALE)
                            for i, kt in enumerate(kts):
                                nc.tensor.matmul(
                                    opsum[0:65, :],
                                    vW[:, kt * 65:(kt + 1) * 65],
                                    pT[:, i * 512:(i + 1) * 512],
                                    start=(kt == 0), stop=(kt == nk - 1),
                                    skip_group_check=True,
                                )
                        o_sb = opath.tile([65, 512], F32, tag="osb")
                        nc.vector.tensor_copy(o_sb, opsum[0:65, :])
                        c0 = COLS[ci] + qb * 512
                        nc.sync.dma_start(
                            out=out[j, :, c0:c0 + 512], in_=o_sb)

    _legalize_waits(nc)
    return nc


_NC = None


def _get_nc():
    global _NC
    if _NC is None:
        _NC = _build_nc()
    return _NC


def make_in_maps(query, key_, value):
    """Host-side prep: per-core slices; Q^T/K^T in [d, seq] layout
    duplicated across both partition halves."""
    qh = query.reshape(B, S, HEADS, HD)
    kh = key_.reshape(B, S, HEADS, HD)
    vh = value.reshape(B, S, HEADS, HD)
    qT = np.ascontiguousarray(qh.transpose(0, 2, 3, 1))  # [B, H, D, S]
    kT = np.ascontiguousarray(kh.transpose(0, 2, 3, 1))
    in_maps = []
    for c in range(N_CORES):
        jobs = [(g // HEADS, g % HEADS) for g in range(4 * c, 4 * c + 4)]
        qt_c = np.empty((JOBS, 128, S), np.float32)
        kt_c = np.empty((JOBS, 128, S), np.float32)
        v_c = np.empty((JOBS, S, HD), np.float32)
        for jj, (b, h) in enumerate(jobs):
            qt_c[jj, 0:64] = qT[b, h]
            qt_c[jj, 64:128] = qT[b, h]
            kt_c[jj, 0:64] = kT[b, h]
            kt_c[jj, 64:128] = kT[b, h]
            v_c[jj] = vh[b, :, h]
        in_maps.append({"qt": qt_c, "kt": kt_c, "v": v_c})
    return in_maps


def assemble_out(results):
    """Host: per-chunk softmax division + overlap-band blending (fp32,
    mirrors the reference's merge), then scatter into [B, S, HIDDEN]."""
    wt = np.linspace(1.0, 0.0, 128).astype(np.float32)  # prev-chunk tail
    wh = np.linspace(0.0, 1.0, 128).astype(np.float32)  # cur-chunk head
    denom = (wt + wh) + np.float32(1e-10)
    a = (wt / denom).astype(np.float32)[:, None]
    bb = (wh / denom).astype(np.float32)[:, None]

    out = np.empty((B, S, HIDDEN), dtype=np.float32)
    for c in range(N_CORES):
        oc = results[c]["out"]  # [4, 65, SQ]
        for jj, g in enumerate(range(4 * c, 4 * c + 4)):
            b, h = g // HEADS, g % HEADS
            full = np.empty((S, HD), np.float32)
            prev_tail = None
            for ci, (q0, lq, k0, lk) in enumerate(CHUNKS):
                off = COLS[ci]
                blk = oc[jj, :, off:off + lq]
                on = (blk[0:64] / blk[64:65]).T  # [lq, 64] normalized
                lo = 0
                if ci > 0:
                    full[q0:q0 + 128] = prev_tail * a + on[0:128] * bb
                    lo = 128
                hi = lq
                if ci < len(CHUNKS) - 1:
                    hi = lq - 128
                    prev_tail = on[lq - 128:lq]
                full[q0 + lo:q0 + hi] = on[lo:hi]
            out[b, :, h * HD:(h + 1) * HD] = full
    return out


def kernel(query, key, value):
    query = np.asarray(query, dtype=np.float32)
    key_ = np.asarray(key, dtype=np.float32)
    value = np.asarray(value, dtype=np.float32)
    nc = _get_nc()
    in_maps = make_in_maps(query, key_, value)
    res = run_bass_kernel_spmd(nc, in_maps, list(range(N_CORES)))
    return assemble_out(res.results)
